# revision 17
# baseline (speedup 1.0000x reference)
"""Trainium2 Bass kernel for the NonLinearTransitionModel neural-ODE.

z_{t+1} = z_t + (dt/NSTEPS) * (tanh([z_t; u] @ W1 + b1) @ W2 + b2), 20 steps.

Sharding: data-parallel over the batch dim (8192 -> 8 x 1024), MLP weights
replicated. Per core the activations are feature-major (features on SBUF
partitions, batch on the free axis) so the mm1 -> tanh -> mm2 chain needs
no per-step transposes; batch is transposed once on entry and once on exit
via PE-transposes.

v3 design (per-step, per-core, steady state, BC=512, 2 chunks):
  PE    : per chunk 8 mm1 (f32r) + 8 mm2 (bf16) N=512 matmuls. The
          constant u-contribution cu = W1u^T u is NOT a matmul: it is
          preloaded into each PSUM bank while the bank is free, and mm1
          accumulates onto it with start=False (hardware has_written
          bits survive from the previous accumulation group).
  ACT   : 8x tanh [128,512] PSUM->SBUF (bias=b1 folded), bf16 output
          feeding mm2; plus 2 cu preloads for the late-freed banks of
          the second chunk.
  DVE   : 4x hb-mul (PSUM src) + 6 cu preloads.
  GpSimd: 4x z += tmp (f32r-tagged output so mm1 consumes z directly).
mm2 writes banks (c,2),(c,3) so the early-deadline banks (0,1) are freed
early (by tanh) and the late-freed banks (2,3, freed by the hb-mul) have
late deadlines (A2/A3 of the next step).
"""

import sys

try:
    import concourse.bass as bass
except ImportError:
    sys.path.insert(0, "/opt/trn_rl_repo")
    import concourse.bass as bass

import numpy as np
import concourse.bacc as bacc
import concourse.mybir as mybir
from concourse import masks, tile
from concourse.bass_utils import run_bass_kernel_spmd

AFT = mybir.ActivationFunctionType
F32 = mybir.dt.float32
R = mybir.dt.float32r
BF16 = mybir.dt.bfloat16

N_CORES = 8
NSTEPS = 20
B, LATENT, U, HIDDEN = 8192, 256, 16, 512
BL = B // N_CORES          # batch rows per core
BC = 512                   # batch columns per chunk (free axis)
NCHUNK = BL // BC          # 2
KIN = LATENT + U           # 272
ML = LATENT // 128         # 2
MH = HIDDEN // 128         # 4

_cache = {}


def _build(add_eng="gdgd", mm2_dt="bf16", b2_nonzero=False, nwarm=20):
    nc = bacc.Bacc(None, target_bir_lowering=False, debug=False)

    zt_d = nc.dram_tensor("zt", [BL, LATENT], R, kind="ExternalInput")
    dt_d = nc.dram_tensor("dt", [BL, 1], F32, kind="ExternalInput")
    ut_d = nc.dram_tensor("ut", [BL, U], R, kind="ExternalInput")
    w1_d = nc.dram_tensor("W1", [KIN, HIDDEN], R, kind="ExternalInput")
    b1_d = nc.dram_tensor("b1", [HIDDEN], F32, kind="ExternalInput")
    w2_d = nc.dram_tensor("W2", [HIDDEN, LATENT], R, kind="ExternalInput")
    b2_d = nc.dram_tensor("b2", [LATENT], F32, kind="ExternalInput")
    out_d = nc.dram_tensor("out", [BL, LATENT], F32, kind="ExternalOutput")

    M2 = BF16 if mm2_dt == "bf16" else R

    with tile.TileContext(nc) as tc:
        with (
            tc.tile_pool(name="const", bufs=1) as cpool,
            tc.tile_pool(name="state", bufs=1) as spool,
            tc.tile_pool(name="stage", bufs=1) as gpool,
            tc.tile_pool(name="tbuf", bufs=4) as tpool,
            tc.tile_pool(name="obuf", bufs=4) as opool,
            tc.tile_pool(name="psum", bufs=1, space="PSUM") as ppool,
        ):
            # -------- persistent PSUM banks: [chunk][m-tile] --------
            pb = [
                [
                    ppool.tile([128, BC], F32, tag=f"pb_{c}_{m}", name=f"pb_{c}_{m}")
                    for m in range(MH)
                ]
                for c in range(NCHUNK)
            ]

            # -------- constants / table warm-up --------
            ident_f = cpool.tile([128, 128], F32, tag="ident_f")
            masks.make_identity(nc, ident_f[:])
            identR = cpool.tile([128, 128], R, tag="identR")
            nc.sync.dma_start(identR[:], ident_f[:].bitcast(R))
            # load the ACT tanh table while DMAs are in flight
            wup = cpool.tile([128, 128], F32, tag="wup")
            nc.scalar.activation(wup[:], ident_f[:], AFT.Tanh)

            # -------- input DMAs, split across the HWDGE/SWDGE rings --------
            # zt halves: block ci holds rows ci*128.. (1KB contiguous each)
            zbs = []
            for c in range(NCHUNK):
                zbc = gpool.tile([128, 4 * LATENT], R, tag=f"zb{c}", name=f"zb{c}")
                eng = nc.sync
                eng.dma_start(
                    zbc[:].rearrange("p (c l) -> p c l", l=LATENT),
                    zt_d.ap()[c * BC : (c + 1) * BC, :].rearrange(
                        "(c p) l -> p c l", p=128
                    ),
                )
                zbs.append(zbc)
            w1ab = cpool.tile([128, 2 * HIDDEN], R, tag="w1ab")
            nc.scalar.dma_start(
                w1ab[:].rearrange("p (k h) -> p k h", h=HIDDEN),
                w1_d.ap()[0 : 2 * 128, :].rearrange("(k p) h -> p k h", p=128),
            )
            w2all = cpool.tile([128, MH * LATENT], F32, tag="w2all")
            nc.scalar.dma_start(
                w2all[:].bitcast(R).rearrange("p (k l) -> p k l", l=LATENT),
                w2_d.ap().rearrange("(k p) l -> p k l", p=128),
            )
            w1u = cpool.tile([U, HIDDEN], R, tag="w1u")
            nc.gpsimd.dma_start(w1u[:], w1_d.ap()[2 * 128 : KIN, :])
            ub = gpool.tile([128, NCHUNK * 4 * U], R, tag="ub", name="ub")
            nc.gpsimd.dma_start(
                ub[:].rearrange("p (c u) -> p c u", u=U),
                ut_d.ap().rearrange("(c p) u -> p c u", p=128),
            )
            b1t = cpool.tile([128, MH], F32, tag="b1t")
            nc.gpsimd.dma_start(b1t[:], b1_d.ap().rearrange("(f p) -> p f", p=128))
            h_row = cpool.tile([1, BL], F32, tag="hrow")
            nc.gpsimd.dma_start(h_row[:], dt_d.ap().rearrange("b o -> o b"))
            if b2_nonzero:
                b2f = cpool.tile([1, LATENT], F32, tag="b2f")
                nc.gpsimd.dma_start(b2f[:], b2_d.ap().unsqueeze(0))
                b2r = cpool.tile([1, LATENT], M2, tag="b2r")
                nc.vector.tensor_copy(b2r[:], b2f[:])
                onesb_f = cpool.tile([1, BC], F32, tag="onesb_f")
                nc.vector.memset(onesb_f[:], 1.0)
                onesb = cpool.tile([1, BC], M2, tag="onesb")
                nc.vector.tensor_copy(onesb[:], onesb_f[:])

            w1a = w1ab[:, 0:HIDDEN]
            w1b = w1ab[:, HIDDEN : 2 * HIDDEN]

            # mm2 weights cast to the mm2 dtype
            if mm2_dt == "bf16":
                w2m = cpool.tile([128, MH * LATENT], BF16, tag="w2m")
                nc.vector.tensor_copy(w2m[:], w2all[:])
            else:
                w2m = w2all  # already f32r-tagged via the DMA bitcast
            w2t = [w2m[:, k * LATENT : (k + 1) * LATENT] for k in range(MH)]

            # -------- PE warm-up stream (HAM) while DMAs land --------
            for i in range(nwarm):
                nc.tensor.transpose(
                    pb[0][0][:, 0:128].bitcast(R), identR[:], identR[:]
                )

            # -------- z transposes: zb -> pb banks -> zts --------
            zts = [
                [
                    spool.tile([128, BC], R, tag=f"z_{c}_{l}", name=f"z_{c}_{l}")
                    for l in range(ML)
                ]
                for c in range(NCHUNK)
            ]
            for c in range(NCHUNK):
                for l in range(ML):
                    bank = pb[c][l]
                    for j in range(BC // 128):
                        nc.tensor.transpose(
                            bank[:, j * 128 : (j + 1) * 128].bitcast(R),
                            zbs[c][:, j * LATENT + l * 128 : j * LATENT + (l + 1) * 128],
                            identR[:],
                        )
                    if (c + l) % 2 == 0:
                        nc.scalar.activation(zts[c][l][:], bank[:], AFT.Copy)
                    else:
                        nc.vector.tensor_copy(zts[c][l][:], bank[:])

            # -------- u transposes: ub -> pb[c][2] low partitions -> uts --------
            uts = []
            for c in range(NCHUNK):
                bank = pb[c][2]
                for j in range(BC // 128):
                    ci = c * (BC // 128) + j
                    nc.tensor.transpose(
                        bank[0:U, j * 128 : (j + 1) * 128].bitcast(R),
                        ub[:, ci * U : (ci + 1) * U],
                        identR[:],
                    )
                ut_c = spool.tile([U, BC], R, tag=f"ut_{c}", name=f"ut_{c}")
                nc.vector.tensor_copy(ut_c[:], bank[0:U, :])
                uts.append(ut_c)

            # -------- hb[p, b] = dt[b] / NSTEPS, replicated over partitions ----
            h_sc = cpool.tile([1, BL], F32, tag="h_sc")
            nc.scalar.activation(h_sc[:], h_row[:], AFT.Copy, scale=1.0 / NSTEPS)
            hb = cpool.tile([128, BL], F32, tag="hb")
            nc.gpsimd.partition_broadcast(hb[:], h_sc[:])

            # -------- cu precompute (all banks; also primes them for t=0) ----
            cus = {}
            for c in range(NCHUNK):
                for m in range(MH):
                    nc.tensor.matmul(
                        pb[c][m][:],
                        w1u[:, bass.ts(m, 128)],
                        uts[c][:],
                        start=True,
                        stop=True,
                    )
                    cu = spool.tile(
                        [128, BC], F32, tag=f"cu_{c}_{m}", name=f"cu_{c}_{m}"
                    )
                    if m % 2 == 0:
                        nc.scalar.activation(cu[:], pb[c][m][:], AFT.Copy)
                    else:
                        nc.vector.tensor_copy(cu[:], pb[c][m][:])
                    cus[(c, m)] = cu

            # ht tiles, persistent per (chunk, m)
            ht = [
                [
                    spool.tile([128, BC], M2, tag=f"h_{c}_{m}", name=f"h_{c}_{m}")
                    for m in range(MH)
                ]
                for c in range(NCHUNK)
            ]

            # ---------------- main loop ----------------
            for t in range(NSTEPS):
                last = t == NSTEPS - 1
                for c in range(NCHUNK):
                    for m in range(MH):
                        nc.tensor.matmul(
                            pb[c][m][:],
                            w1a[:, bass.ts(m, 128)],
                            zts[c][0][:],
                            start=False,
                            stop=False,
                            skip_group_check=True,
                        )
                    for m in range(MH):
                        nc.tensor.matmul(
                            pb[c][m][:],
                            w1b[:, bass.ts(m, 128)],
                            zts[c][1][:],
                            start=False,
                            stop=True,
                            skip_group_check=True,
                        )
                    for m in range(MH):
                        nc.scalar.activation(
                            ht[c][m][:], pb[c][m][:], AFT.Tanh,
                            bias=b1t[:, m : m + 1],
                        )
                    if not last:
                        # early-freed banks (tanh was the last reader)
                        nc.vector.tensor_copy(pb[c][0][:], cus[(c, 0)][:])
                        nc.vector.tensor_copy(pb[c][1][:], cus[(c, 1)][:])
                for c in range(NCHUNK):
                    cs = bass.ts(c, BC)
                    for l in range(ML):
                        p2 = pb[c][2 + l]
                        for k in range(MH):
                            nc.tensor.matmul(
                                p2[:],
                                w2t[k][:, bass.ts(l, 128)],
                                ht[c][k][:],
                                start=(k == 0),
                                stop=(k == MH - 1) and not b2_nonzero,
                            )
                        if b2_nonzero:
                            nc.tensor.matmul(
                                p2[:], b2r[:, bass.ts(l, 128)],
                                onesb[:], start=False, stop=True,
                            )
                        tmp = tpool.tile([128, BC], F32, tag="tmp", bufs=4)
                        nc.vector.tensor_mul(tmp[:], p2[:], hb[:, cs])
                        # z += tmp: per-tile engine choice. The first mm1
                        # matmul of the next step waits (hoisted sem) on the
                        # whole z-update of its chunk, so both tiles' chains
                        # must finish early; GP ops are 2x slower than DVE.
                        on_gp = add_eng[c * ML + l] == "g"
                        if on_gp:
                            nc.gpsimd.tensor_add(
                                zts[c][l][:], zts[c][l][:].bitcast(F32), tmp[:]
                            )
                        else:
                            nc.vector.tensor_add(
                                zts[c][l][:], zts[c][l][:].bitcast(F32), tmp[:]
                            )
                        if not last:
                            # late-freed bank (the hb-mul was the last reader)
                            nc.vector.tensor_copy(
                                pb[c][2 + l][:], cus[(c, 2 + l)][:]
                            )

            # ---------------- epilogue: transpose back, store ----------------
            for c in range(NCHUNK):
                for half in range(2):
                    bank = pb[c][half]
                    for jj in range(2):
                        j = half * 2 + jj
                        for l in range(ML):
                            nc.tensor.transpose(
                                bank[:, (jj * ML + l) * 128 : (jj * ML + l + 1) * 128].bitcast(R),
                                zts[c][l][:, j * 128 : (j + 1) * 128],
                                identR[:],
                            )
                    zo = opool.tile([128, 2 * LATENT], F32, tag="zo", bufs=4)
                    if half == 0:
                        nc.scalar.activation(zo[:], bank[:], AFT.Copy)
                    else:
                        nc.vector.tensor_copy(zo[:], bank[:])
                    r0 = (c * 4 + half * 2) * 128
                    nc.sync.dma_start(
                        out_d.ap()[r0 : r0 + 256, :].rearrange(
                            "(two p) l -> p two l", p=128
                        ),
                        zo[:].rearrange("p (two l) -> p two l", l=LATENT),
                    )

    nc.compile()
    return nc


def _get_nc(add_eng, mm2_dt, b2_nonzero):
    key = (add_eng, mm2_dt, b2_nonzero)
    if key not in _cache:
        _cache[key] = _build(add_eng, mm2_dt, b2_nonzero)
    return _cache[key]


def _run(inputs, add_eng="gdgd", mm2_dt="bf16", trace=False):
    zt = np.ascontiguousarray(inputs["zt"], dtype=np.float32)
    dt = np.ascontiguousarray(inputs["dt"], dtype=np.float32)
    ut = np.ascontiguousarray(inputs["ut"], dtype=np.float32)
    W1 = np.ascontiguousarray(inputs["W1"], dtype=np.float32)
    b1 = np.ascontiguousarray(inputs["b1"], dtype=np.float32)
    W2 = np.ascontiguousarray(inputs["W2"], dtype=np.float32)
    b2 = np.ascontiguousarray(inputs["b2"], dtype=np.float32)

    b2_nonzero = bool(np.any(b2))
    nc = _get_nc(add_eng, mm2_dt, b2_nonzero)

    in_maps = []
    for i in range(N_CORES):
        sl = slice(i * BL, (i + 1) * BL)
        in_maps.append(
            {
                "zt": zt[sl],
                "dt": dt[sl],
                "ut": ut[sl],
                "W1": W1,
                "b1": b1,
                "W2": W2,
                "b2": b2,
            }
        )
    res = run_bass_kernel_spmd(nc, in_maps, list(range(N_CORES)), trace=trace)
    out = np.concatenate([res.results[i]["out"] for i in range(N_CORES)], axis=0)
    return out, res


def kernel(**inputs):
    out, _ = _run(inputs, add_eng="gdgd", mm2_dt="bf16")
    return out


# revision 20
# speedup vs baseline: 1.3037x; 1.3037x over previous
"""Trainium2 Bass kernel for the NonLinearTransitionModel neural-ODE.

z_{t+1} = z_t + (dt/NSTEPS) * (tanh([z_t; u] @ W1 + b1) @ W2 + b2), 20 steps.

Sharding: data-parallel over the batch dim (8192 -> 8 x 1024), MLP weights
replicated. Per core the activations are feature-major (features on SBUF
partitions, batch on the free axis) so the mm1 -> tanh -> mm2 chain needs
no per-step transposes; batch is transposed once on entry and once on exit
via PE-transposes.

v3 design (per-step, per-core, steady state, BC=512, 2 chunks):
  PE    : per chunk 8 mm1 (f32r) + 8 mm2 (bf16) N=512 matmuls. The
          constant u-contribution cu = W1u^T u is NOT a matmul: it is
          preloaded into each PSUM bank while the bank is free, and mm1
          accumulates onto it with start=False (hardware has_written
          bits survive from the previous accumulation group).
  ACT   : 8x tanh [128,512] PSUM->SBUF (bias=b1 folded), bf16 output
          feeding mm2; plus 2 cu preloads for the late-freed banks of
          the second chunk.
  DVE   : 4x hb-mul (PSUM src) + 6 cu preloads.
  GpSimd: 4x z += tmp (f32r-tagged output so mm1 consumes z directly).
mm2 writes banks (c,2),(c,3) so the early-deadline banks (0,1) are freed
early (by tanh) and the late-freed banks (2,3, freed by the hb-mul) have
late deadlines (A2/A3 of the next step).
"""

import sys

try:
    import concourse.bass as bass
except ImportError:
    sys.path.insert(0, "/opt/trn_rl_repo")
    import concourse.bass as bass

import numpy as np
import concourse.bacc as bacc
import concourse.mybir as mybir
from concourse import masks, tile
from concourse.bass_utils import run_bass_kernel_spmd

AFT = mybir.ActivationFunctionType
F32 = mybir.dt.float32
R = mybir.dt.float32r
BF16 = mybir.dt.bfloat16

N_CORES = 8
NSTEPS = 20
B, LATENT, U, HIDDEN = 8192, 256, 16, 512
BL = B // N_CORES          # batch rows per core
BC = 512                   # batch columns per chunk (free axis)
NCHUNK = BL // BC          # 2
KIN = LATENT + U           # 272
ML = LATENT // 128         # 2
MH = HIDDEN // 128         # 4

_cache = {}


def _build(add_eng="gdgd", mm2_dt="bf16", b2_nonzero=False, nwarm=20):
    nc = bacc.Bacc(None, target_bir_lowering=False, debug=False)

    zt_d = nc.dram_tensor("zt", [BL, LATENT], R, kind="ExternalInput")
    dt_d = nc.dram_tensor("dt", [BL, 1], F32, kind="ExternalInput")
    ut_d = nc.dram_tensor("ut", [BL, U], R, kind="ExternalInput")
    w1_d = nc.dram_tensor("W1", [KIN, HIDDEN], R, kind="ExternalInput")
    b1_d = nc.dram_tensor("b1", [HIDDEN], F32, kind="ExternalInput")
    w2_d = nc.dram_tensor("W2", [HIDDEN, LATENT], R, kind="ExternalInput")
    b2_d = nc.dram_tensor("b2", [LATENT], F32, kind="ExternalInput")
    out_d = nc.dram_tensor("out", [BL, LATENT], F32, kind="ExternalOutput")

    M2 = BF16 if mm2_dt == "bf16" else R

    with tile.TileContext(nc) as tc:
        with (
            tc.tile_pool(name="const", bufs=1) as cpool,
            tc.tile_pool(name="state", bufs=1) as spool,
            tc.tile_pool(name="stage", bufs=1) as gpool,
            tc.tile_pool(name="tbuf", bufs=4) as tpool,
            tc.tile_pool(name="obuf", bufs=4) as opool,
            tc.tile_pool(name="psum", bufs=1, space="PSUM") as ppool,
        ):
            # -------- persistent PSUM banks: [chunk][m-tile] --------
            pb = [
                [
                    ppool.tile([128, BC], F32, tag=f"pb_{c}_{m}", name=f"pb_{c}_{m}")
                    for m in range(MH)
                ]
                for c in range(NCHUNK)
            ]

            # -------- constants / table warm-up --------
            ident_f = cpool.tile([128, 128], F32, tag="ident_f")
            masks.make_identity(nc, ident_f[:])
            identR = cpool.tile([128, 128], R, tag="identR")
            nc.sync.dma_start(identR[:], ident_f[:].bitcast(R))
            # load the ACT tanh table while DMAs are in flight
            wup = cpool.tile([128, 128], F32, tag="wup")
            nc.scalar.activation(wup[:], ident_f[:], AFT.Tanh)

            # -------- input DMAs, split across the HWDGE/SWDGE rings --------
            # zt halves: block ci holds rows ci*128.. (1KB contiguous each)
            zbs = []
            for c in range(NCHUNK):
                zbc = gpool.tile([128, 4 * LATENT], R, tag=f"zb{c}", name=f"zb{c}")
                eng = nc.sync
                eng.dma_start(
                    zbc[:].rearrange("p (c l) -> p c l", l=LATENT),
                    zt_d.ap()[c * BC : (c + 1) * BC, :].rearrange(
                        "(c p) l -> p c l", p=128
                    ),
                )
                zbs.append(zbc)
            w1ab = cpool.tile([128, 2 * HIDDEN], R, tag="w1ab")
            nc.scalar.dma_start(
                w1ab[:].rearrange("p (k h) -> p k h", h=HIDDEN),
                w1_d.ap()[0 : 2 * 128, :].rearrange("(k p) h -> p k h", p=128),
            )
            w2all = cpool.tile([128, MH * LATENT], F32, tag="w2all")
            nc.scalar.dma_start(
                w2all[:].bitcast(R).rearrange("p (k l) -> p k l", l=LATENT),
                w2_d.ap().rearrange("(k p) l -> p k l", p=128),
            )
            w1u = cpool.tile([U, HIDDEN], R, tag="w1u")
            nc.gpsimd.dma_start(w1u[:], w1_d.ap()[2 * 128 : KIN, :])
            ub = gpool.tile([128, NCHUNK * 4 * U], R, tag="ub", name="ub")
            nc.gpsimd.dma_start(
                ub[:].rearrange("p (c u) -> p c u", u=U),
                ut_d.ap().rearrange("(c p) u -> p c u", p=128),
            )
            b1t = cpool.tile([128, MH], F32, tag="b1t")
            nc.gpsimd.dma_start(b1t[:], b1_d.ap().rearrange("(f p) -> p f", p=128))
            h_row = cpool.tile([1, BL], F32, tag="hrow")
            nc.gpsimd.dma_start(h_row[:], dt_d.ap().rearrange("b o -> o b"))
            if b2_nonzero:
                b2f = cpool.tile([1, LATENT], F32, tag="b2f")
                nc.gpsimd.dma_start(b2f[:], b2_d.ap().unsqueeze(0))
                b2r = cpool.tile([1, LATENT], M2, tag="b2r")
                nc.vector.tensor_copy(b2r[:], b2f[:])
                onesb_f = cpool.tile([1, BC], F32, tag="onesb_f")
                nc.vector.memset(onesb_f[:], 1.0)
                onesb = cpool.tile([1, BC], M2, tag="onesb")
                nc.vector.tensor_copy(onesb[:], onesb_f[:])

            w1a = w1ab[:, 0:HIDDEN]
            w1b = w1ab[:, HIDDEN : 2 * HIDDEN]

            # mm2 weights cast to the mm2 dtype
            if mm2_dt == "bf16":
                w2m = cpool.tile([128, MH * LATENT], BF16, tag="w2m")
                nc.vector.tensor_copy(w2m[:], w2all[:])
            else:
                w2m = w2all  # already f32r-tagged via the DMA bitcast
            w2t = [w2m[:, k * LATENT : (k + 1) * LATENT] for k in range(MH)]

            # -------- PE warm-up stream (HAM) while DMAs land --------
            for i in range(nwarm):
                nc.tensor.transpose(
                    pb[0][0][:, 0:128].bitcast(R), identR[:], identR[:]
                )

            # -------- z transposes: zb -> pb banks -> zts --------
            zts = [
                [
                    spool.tile([128, BC], R, tag=f"z_{c}_{l}", name=f"z_{c}_{l}")
                    for l in range(ML)
                ]
                for c in range(NCHUNK)
            ]
            for c in range(NCHUNK):
                for l in range(ML):
                    bank = pb[c][l]
                    for j in range(BC // 128):
                        nc.tensor.transpose(
                            bank[:, j * 128 : (j + 1) * 128].bitcast(R),
                            zbs[c][:, j * LATENT + l * 128 : j * LATENT + (l + 1) * 128],
                            identR[:],
                        )
                    if (c + l) % 2 == 0:
                        nc.scalar.activation(zts[c][l][:], bank[:], AFT.Copy)
                    else:
                        nc.vector.tensor_copy(zts[c][l][:], bank[:])

            # -------- u transposes: ub -> pb[c][2] low partitions -> uts --------
            uts = []
            for c in range(NCHUNK):
                bank = pb[c][2]
                for j in range(BC // 128):
                    ci = c * (BC // 128) + j
                    nc.tensor.transpose(
                        bank[0:U, j * 128 : (j + 1) * 128].bitcast(R),
                        ub[:, ci * U : (ci + 1) * U],
                        identR[:],
                    )
                ut_c = spool.tile([U, BC], R, tag=f"ut_{c}", name=f"ut_{c}")
                nc.vector.tensor_copy(ut_c[:], bank[0:U, :])
                uts.append(ut_c)

            # -------- hb[p, b] = dt[b] / NSTEPS, replicated over partitions ----
            h_sc = cpool.tile([1, BL], F32, tag="h_sc")
            nc.scalar.activation(h_sc[:], h_row[:], AFT.Copy, scale=1.0 / NSTEPS)
            hb = cpool.tile([128, BL], F32, tag="hb")
            nc.gpsimd.partition_broadcast(hb[:], h_sc[:])

            # -------- cu precompute (all banks; also primes them for t=0) ----
            cus = {}
            for c in range(NCHUNK):
                for m in range(MH):
                    nc.tensor.matmul(
                        pb[c][m][:],
                        w1u[:, bass.ts(m, 128)],
                        uts[c][:],
                        start=True,
                        stop=True,
                    )
                    cu = spool.tile(
                        [128, BC], F32, tag=f"cu_{c}_{m}", name=f"cu_{c}_{m}"
                    )
                    if m % 2 == 0:
                        nc.scalar.activation(cu[:], pb[c][m][:], AFT.Copy)
                    else:
                        nc.vector.tensor_copy(cu[:], pb[c][m][:])
                    cus[(c, m)] = cu

            # ht tiles, persistent per (chunk, m)
            ht = [
                [
                    spool.tile([128, BC], M2, tag=f"h_{c}_{m}", name=f"h_{c}_{m}")
                    for m in range(MH)
                ]
                for c in range(NCHUNK)
            ]

            # ---------------- main loop ----------------
            for t in range(NSTEPS):
                last = t == NSTEPS - 1
                for c in range(NCHUNK):
                    for m in range(MH):
                        nc.tensor.matmul(
                            pb[c][m][:],
                            w1a[:, bass.ts(m, 128)],
                            zts[c][0][:],
                            start=False,
                            stop=False,
                            skip_group_check=True,
                        )
                    for m in range(MH):
                        nc.tensor.matmul(
                            pb[c][m][:],
                            w1b[:, bass.ts(m, 128)],
                            zts[c][1][:],
                            start=False,
                            stop=True,
                            skip_group_check=True,
                        )
                    for m in range(MH):
                        nc.scalar.activation(
                            ht[c][m][:], pb[c][m][:], AFT.Tanh,
                            bias=b1t[:, m : m + 1],
                        )
                if not last:
                    # early-freed banks (tanh was the last reader): preload on
                    # ACT *after* both chunks' tanhs so the tanh pipeline is
                    # never blocked; these have no cross-engine deps and the
                    # A0/A1 deadlines are a full mm2-phase away.
                    for c in range(NCHUNK):
                        nc.scalar.activation(pb[c][0][:], cus[(c, 0)][:], AFT.Copy)
                        nc.scalar.activation(pb[c][1][:], cus[(c, 1)][:], AFT.Copy)
                for c in range(NCHUNK):
                    cs = bass.ts(c, BC)
                    for l in range(ML):
                        p2 = pb[c][2 + l]
                        for k in range(MH):
                            nc.tensor.matmul(
                                p2[:],
                                w2t[k][:, bass.ts(l, 128)],
                                ht[c][k][:],
                                start=(k == 0),
                                stop=(k == MH - 1) and not b2_nonzero,
                            )
                        if b2_nonzero:
                            nc.tensor.matmul(
                                p2[:], b2r[:, bass.ts(l, 128)],
                                onesb[:], start=False, stop=True,
                            )
                        tmp = tpool.tile([128, BC], F32, tag="tmp", bufs=4)
                        nc.vector.tensor_mul(tmp[:], p2[:], hb[:, cs])
                        # z += tmp: per-tile engine choice. The first mm1
                        # matmul of the next step waits (hoisted sem) on the
                        # whole z-update of its chunk, so both tiles' chains
                        # must finish early; GP ops are 2x slower than DVE.
                        on_gp = add_eng[c * ML + l] == "g"
                        if on_gp:
                            nc.gpsimd.tensor_add(
                                zts[c][l][:], zts[c][l][:].bitcast(F32), tmp[:]
                            )
                        else:
                            nc.vector.tensor_add(
                                zts[c][l][:], zts[c][l][:].bitcast(F32), tmp[:]
                            )
                        if not last:
                            # late-freed bank (the hb-mul was the last reader).
                            # Engine per the hand schedule: (c0,2)->GP,
                            # (c1,2)->ACT (loose deadline), (*,3)->DVE.
                            bank, cu = pb[c][2 + l][:], cus[(c, 2 + l)][:]
                            if l == 0 and c == 1:
                                # ACT: loose deadline, keeps DVE free for the
                                # c1 z-chain (GPSIMD cannot write PSUM).
                                nc.scalar.activation(bank, cu, AFT.Copy)
                            else:
                                nc.vector.tensor_copy(bank, cu)

            # ---------------- epilogue: transpose back, store ----------------
            for c in range(NCHUNK):
                for half in range(2):
                    bank = pb[c][half]
                    for jj in range(2):
                        j = half * 2 + jj
                        for l in range(ML):
                            nc.tensor.transpose(
                                bank[:, (jj * ML + l) * 128 : (jj * ML + l + 1) * 128].bitcast(R),
                                zts[c][l][:, j * 128 : (j + 1) * 128],
                                identR[:],
                            )
                    zo = opool.tile([128, 2 * LATENT], F32, tag="zo", bufs=4)
                    if half == 0:
                        nc.scalar.activation(zo[:], bank[:], AFT.Copy)
                    else:
                        nc.vector.tensor_copy(zo[:], bank[:])
                    r0 = (c * 4 + half * 2) * 128
                    nc.sync.dma_start(
                        out_d.ap()[r0 : r0 + 256, :].rearrange(
                            "(two p) l -> p two l", p=128
                        ),
                        zo[:].rearrange("p (two l) -> p two l", l=LATENT),
                    )

    nc.compile()
    return nc


def _get_nc(add_eng, mm2_dt, b2_nonzero):
    key = (add_eng, mm2_dt, b2_nonzero)
    if key not in _cache:
        _cache[key] = _build(add_eng, mm2_dt, b2_nonzero)
    return _cache[key]


def _run(inputs, add_eng="gdgd", mm2_dt="bf16", trace=False):
    zt = np.ascontiguousarray(inputs["zt"], dtype=np.float32)
    dt = np.ascontiguousarray(inputs["dt"], dtype=np.float32)
    ut = np.ascontiguousarray(inputs["ut"], dtype=np.float32)
    W1 = np.ascontiguousarray(inputs["W1"], dtype=np.float32)
    b1 = np.ascontiguousarray(inputs["b1"], dtype=np.float32)
    W2 = np.ascontiguousarray(inputs["W2"], dtype=np.float32)
    b2 = np.ascontiguousarray(inputs["b2"], dtype=np.float32)

    b2_nonzero = bool(np.any(b2))
    nc = _get_nc(add_eng, mm2_dt, b2_nonzero)

    in_maps = []
    for i in range(N_CORES):
        sl = slice(i * BL, (i + 1) * BL)
        in_maps.append(
            {
                "zt": zt[sl],
                "dt": dt[sl],
                "ut": ut[sl],
                "W1": W1,
                "b1": b1,
                "W2": W2,
                "b2": b2,
            }
        )
    res = run_bass_kernel_spmd(nc, in_maps, list(range(N_CORES)), trace=trace)
    out = np.concatenate([res.results[i]["out"] for i in range(N_CORES)], axis=0)
    return out, res


def kernel(**inputs):
    out, _ = _run(inputs, add_eng="gdgd", mm2_dt="bf16")
    return out
